# revision 1
# baseline (speedup 1.0000x reference)
"""CartesianMACE message-passing kernel for 8 Trainium2 NeuronCores.

Self-contained: kernel(**inputs) takes the FULL inputs and returns the FULL
[G, OUT] output. Internally it shards edges by receiver across the 8 cores
(receiver-contiguous blocks of <=128 nodes / <=2048 edges), runs a Bass/Tile
SPMD kernel (segment-sum via one-hot matmuls into PSUM, node-level channel
mixing on the PE, AllGather of node features between the two layers,
per-graph pooling + AllReduce + prediction head on-device), and returns
core 0's output.

Host-side work is limited to index/layout preparation (edge sorting and
blocking, gather-index wrapping, weight transposes/folds); all O(E)/O(N)
numerical work runs on the NeuronCores.
"""
import sys
sys.path.insert(0, "/opt/trn_rl_repo")
import numpy as np
import concourse.bass as bass
import concourse.bacc as bacc
import concourse.mybir as mybir
from concourse import tile, masks

f32 = mybir.dt.float32
i16 = mybir.dt.int16
i32 = mybir.dt.int32
AF = mybir.ActivationFunctionType
ALU = mybir.AluOpType

PI = float(np.pi)
TWO_PI = float(2 * np.pi)
MAGIC = 12582912.0  # 1.5*2^23: add/sub rounds fp32 to nearest int
CW1 = float(np.float32(6.28125))
CW2 = float(np.float32(TWO_PI - CW1))
CW3 = float(TWO_PI - CW1 - float(np.float32(TWO_PI - CW1)))
INV2PI = float(1.0 / TWO_PI)

NCORES = 8
C = 16
L = 2
RANKS = 3
EPB = 2048
TPB = 16
BLKN = 128
CH = 8
NCHUNK = TPB // CH
F160 = 160
FPAD = 192
POSPAD = 64
KMAP = [0, 1, 1, 1, 2, 2, 2, 2, 2, 2]
SYM_A = np.array([0, 0, 0, 1, 1, 2])
SYM_B = np.array([0, 1, 2, 1, 2, 2])


def _wrap_idx(idx, cols):
    n = idx.shape[0]
    w = np.zeros((16, cols), dtype=np.int16)
    w[np.arange(n) % 16, np.arange(n) // 16] = idx.astype(np.int16)
    return np.tile(w, (8, 1))


def _host_prep(pos, emb_weight, ab_w, ws_w, channel_w, message_w, pred_W, pred_b,
               atoms, edge_index, batch):
    N = pos.shape[0]
    OUT = pred_W.shape[1]
    src = np.asarray(edge_index[0], dtype=np.int64)
    rcv = np.asarray(edge_index[1], dtype=np.int64)

    deg = np.bincount(rcv, minlength=N)
    assert deg.max() <= EPB

    blocks = []
    n0, cur = 0, 0
    for n in range(N):
        d = deg[n]
        if n > n0 and (n - n0 >= BLKN or cur + d > EPB):
            blocks.append((n0, n))
            n0, cur = n, 0
        cur += d
    blocks.append((n0, N))
    B = (len(blocks) + NCORES - 1) // NCORES
    while len(blocks) < NCORES * B:
        blocks.append((N, N))
    NSLOT = NCORES * B * BLKN
    assert NSLOT < 32768  # int16 gather indices

    node_slot = np.full(N, -1, dtype=np.int64)
    for bi, (a, b) in enumerate(blocks):
        node_slot[a:b] = bi * BLKN + np.arange(b - a)

    order = np.argsort(rcv, kind="stable")
    rs, ss = rcv[order], src[order]
    starts = np.searchsorted(rs, [a for a, _ in blocks])
    ends = np.searchsorted(rs, [b for _, b in blocks])

    esnd = np.zeros((NCORES, B, EPB), dtype=np.int64)
    ercv = np.ones((NCORES, B, EPB), dtype=np.int64)
    eslot = np.zeros((NCORES, B, EPB), dtype=np.int64)
    rv_adj = np.full((NCORES, B, EPB), -1.0, dtype=np.float32)
    for bi, (a, b) in enumerate(blocks):
        k, bb = divmod(bi, B)
        e0, e1 = starts[bi], ends[bi]
        ne = e1 - e0
        esnd[k, bb, :ne] = ss[e0:e1]
        ercv[k, bb, :ne] = rs[e0:e1]
        eslot[k, bb, :ne] = node_slot[ss[e0:e1]]
        rv_adj[k, bb, :ne] = (rs[e0:e1] - a).astype(np.float32)

    rv_dev = np.transpose(rv_adj.reshape(NCORES, B, TPB, 128), (0, 3, 1, 2)).copy()

    pos_idx = np.zeros((NCORES, B * 128, 256), dtype=np.int16)
    slot_idxb = np.zeros((NCORES, 128, B * 128), dtype=np.int16)
    for k in range(NCORES):
        for bb in range(B):
            tok = np.concatenate([esnd[k, bb], ercv[k, bb]])
            pos_idx[k, bb * 128:(bb + 1) * 128, :] = _wrap_idx(tok, 256)
            slot_idxb[k, :, bb * 128:(bb + 1) * 128] = _wrap_idx(eslot[k, bb], 128)

    batch_slot = np.full((NCORES, 128, B), -1.0, dtype=np.float32)
    for bi, (a, b) in enumerate(blocks):
        k, bb = divmod(bi, B)
        batch_slot[k, :b - a, bb] = batch[a:b].astype(np.float32)

    pos_pad = np.zeros((N, POSPAD), dtype=np.float32)
    pos_pad[:, :3] = pos
    npi = np.tile((np.arange(1, C + 1) * PI).astype(np.float32)[None, :], (128, 1))

    emb0 = np.asarray(emb_weight[0], dtype=np.float32)
    abwT = np.zeros((L, RANKS, C, C), dtype=np.float32)
    for c in range(RANKS):
        abwT[0, c] = (ab_w[0, c] * emb0[None, :]).T
        abwT[1, c] = (ab_w[1, c] * (2.0 if c == 0 else 1.0)).T
    abwT32 = np.zeros((L, 5, 32, 32), dtype=np.float32)
    for l in range(L):
        for g in range(5):
            abwT32[l, g, 0:16, 0:16] = abwT[l, KMAP[2 * g]]
            abwT32[l, g, 16:32, 16:32] = abwT[l, KMAP[2 * g + 1]]
    abw_dev = np.transpose(abwT32, (2, 0, 1, 3)).reshape(32, L * 5 * 32).copy()
    rep16 = np.zeros((16, 128), dtype=np.float32)
    rep16[np.arange(128) % 16, np.arange(128)] = 1.0

    def vec_for(l, fn):
        v = np.zeros(160, dtype=np.float32)
        for k in range(10):
            v[k * 16:(k + 1) * 16] = fn(l, KMAP[k])
        return v
    wsv = np.zeros((L, 160, 5), dtype=np.float32)
    for l in range(L):
        wsv[l, :, 0] = vec_for(l, lambda l_, c: ws_w[l_, 0, c])
        wsv[l, :, 1] = vec_for(l, lambda l_, c: ws_w[l_, 1, c])
        wsv[l, :, 2] = vec_for(l, lambda l_, c: channel_w[l_, c].sum(0))
        wsv[l, :, 3] = vec_for(l, lambda l_, c: message_w[l_, c].sum(0))
    wsv[0, 0:16, 4] = emb0 * channel_w[0, 0].sum(0)
    wsv_a = np.transpose(wsv[:, 0:128, :], (1, 0, 2)).reshape(128, L * 5).copy()
    wsv_b = np.transpose(wsv[:, 128:160, :], (1, 0, 2)).reshape(32, L * 5).copy()

    predW_eff = np.zeros((160, OUT), dtype=np.float32)
    predW_eff[0:16] = pred_W[0:16]
    for x in range(3):
        for ch in range(C):
            predW_eff[16 + x * 16 + ch] = pred_W[16 + ch * 3 + x]
    for s in range(6):
        a_, b_ = SYM_A[s], SYM_B[s]
        for ch in range(C):
            w = pred_W[64 + ch * 9 + a_ * 3 + b_].copy()
            if a_ != b_:
                w = w + pred_W[64 + ch * 9 + b_ * 3 + a_]
            predW_eff[64 + s * 16 + ch] = w
    predb_rep = np.tile(np.asarray(pred_b, dtype=np.float32)[None, :], (64, 1))

    return dict(N=N, B=B, NSLOT=NSLOT, pos_idx=pos_idx, slot_idxb=slot_idxb,
                rv_dev=rv_dev, batch_slot=batch_slot, pos_pad=pos_pad, npi=npi,
                abw_dev=abw_dev, rep16=rep16, wsv_a=wsv_a, wsv_b=wsv_b,
                predW1=predW_eff[0:128].copy(), predW2=predW_eff[128:160].copy(),
                predb_rep=predb_rep)


def _build(prep, n_cores=NCORES):
    B = prep["B"]
    NSLOT = prep["NSLOT"]
    W = B * 128

    nc = bacc.Bacc("TRN2", target_bir_lowering=False, debug=False,
                   num_devices=n_cores)

    pos_pad = nc.dram_tensor("pos_pad", [prep["N"], POSPAD], f32, kind="ExternalInput")
    npi_in = nc.dram_tensor("npi", [128, C], f32, kind="ExternalInput")
    pidx_in = nc.dram_tensor("pos_idx", [B * 128, 256], i16, kind="ExternalInput")
    slot_in = nc.dram_tensor("slot_idxb", [128, B * 128], i16, kind="ExternalInput")
    rv_in = nc.dram_tensor("rv_dev", [128, B * TPB], f32, kind="ExternalInput")
    batch_in = nc.dram_tensor("batch_slot", [128, B], f32, kind="ExternalInput")
    abw_in = nc.dram_tensor("abw_dev", [32, L * 5 * 32], f32, kind="ExternalInput")
    rep_in = nc.dram_tensor("rep16", [16, 128], f32, kind="ExternalInput")
    wsva_in = nc.dram_tensor("wsv_a", [128, L * 5], f32, kind="ExternalInput")
    wsvb_in = nc.dram_tensor("wsv_b", [32, L * 5], f32, kind="ExternalInput")
    pw1_in = nc.dram_tensor("predW1", [128, 2], f32, kind="ExternalInput")
    pw2_in = nc.dram_tensor("predW2", [32, 2], f32, kind="ExternalInput")
    pb_in = nc.dram_tensor("predb", [64, 2], f32, kind="ExternalInput")
    out_t = nc.dram_tensor("out", [64, 2], f32, kind="ExternalOutput")

    with tile.TileContext(nc) as tc:
        with (
            tc.tile_pool(name="const", bufs=1) as cpool,
            tc.tile_pool(name="geo", bufs=1) as geo,
            tc.tile_pool(name="st", bufs=2) as st,
            tc.tile_pool(name="st3", bufs=3) as st3,
            tc.tile_pool(name="wide", bufs=1) as wd,
            tc.tile_pool(name="psA", bufs=2, space="PSUM") as psA,
            tc.tile_pool(name="psT", bufs=2, space="PSUM") as psT,
            tc.tile_pool(name="psW", bufs=1, space="PSUM") as psW,
            tc.tile_pool(name="dram", bufs=1, space="DRAM") as dram,
        ):
            def ct(shape, name, dt=f32):
                return cpool.tile(shape, dt, name=name, tag=name)

            npi = ct([128, C], "npi_t")
            nc.sync.dma_start(npi[:], npi_in[:])
            rv_all = ct([128, B * TPB], "rv_all")
            nc.sync.dma_start(rv_all[:], rv_in[:])
            batch_t = ct([128, B], "batch_t")
            nc.sync.dma_start(batch_t[:], batch_in[:])
            abw_t = ct([32, L * 5 * 32], "abw_t")
            nc.sync.dma_start(abw_t[:], abw_in[:])
            rep_t = ct([16, 128], "rep_t")
            nc.sync.dma_start(rep_t[:], rep_in[:])
            wsva = ct([128, L * 5], "wsva_t")
            nc.sync.dma_start(wsva[:], wsva_in[:])
            wsvb = ct([32, L * 5], "wsvb_t")
            nc.sync.dma_start(wsvb[:], wsvb_in[:])
            pw1 = ct([128, 2], "pw1_t")
            nc.sync.dma_start(pw1[:], pw1_in[:])
            pw2 = ct([32, 2], "pw2_t")
            nc.sync.dma_start(pw2[:], pw2_in[:])
            pb = ct([64, 2], "pb_t")
            nc.sync.dma_start(pb[:], pb_in[:])
            slotx = ct([128, B * 128], "slotx", i16)
            nc.sync.dma_start(slotx[:], slot_in[:])
            iota_i = ct([128, 128], "iota_i", i32)
            nc.gpsimd.iota(iota_i[:], pattern=[[1, 128]], base=0, channel_multiplier=0)
            iota_f = ct([128, 128], "iota_f")
            nc.vector.tensor_copy(iota_f[:], iota_i[:])
            ident = ct([128, 128], "ident")
            masks.make_identity(nc, ident[:])

            rad_t = [geo.tile([128, TPB, C], f32, name=f"rad{b}", tag=f"rad{b}")
                     for b in range(B)]
            ruu_t = [geo.tile([128, TPB, 9], f32, name=f"ruu{b}", tag=f"ruu{b}")
                     for b in range(B)]
            d2_all = geo.tile([128, B * TPB], f32, name="d2_all", tag="d2_all")
            invd_all = geo.tile([128, B * TPB], f32, name="invd_all", tag="invd_all")

            def wt(shape, name):
                return wd.tile(shape, f32, name=name, tag=name)
            ATg = [wt([32, W], f"ATg{g}") for g in range(4)]
            bPack = wt([32, 3 * W], "bPack")
            ATg.append(bPack[:, 0:W])
            Fa = wt([128, W], "Fa")
            Fb = bPack[:, W:2 * W]
            hTa = wt([128, W], "hTa")
            hTb = bPack[:, 2 * W:3 * W]

            h_local = dram.tile([B * 128, FPAD], f32, name="h_local", tag="h_local")
            h_full = dram.tile([NSLOT, FPAD], f32, name="h_full", tag="h_full",
                               addr_space="Shared")
            ar_in = dram.tile([64, F160], f32, name="ar_in", tag="ar_in")
            ar_out = dram.tile([64, F160], f32, name="ar_out", tag="ar_out")

            # ---- geometry ----
            d_all = geo.tile([128, B * TPB], f32, name="d_all", tag="d_all")
            for b in range(B):
                pidx = st.tile([128, 256], i16, name="pidx", tag="pidx")
                nc.sync.dma_start(pidx[:], pidx_in[b * 128:(b + 1) * 128, :])
                pp = st.tile([128, 32, POSPAD], f32, name="pp", tag="hgb", bufs=2)
                for qk in range(4):
                    nc.gpsimd.dma_gather(pp[:, 8 * qk:8 * (qk + 1), :], pos_pad[:, :],
                                         pidx[:, 64 * qk:64 * (qk + 1)],
                                         num_idxs=1024, num_idxs_reg=1024,
                                         elem_size=POSPAD)
                bsl = slice(b * TPB, (b + 1) * TPB)
                nc.vector.tensor_tensor(ruu_t[b][:, :, 0:3], pp[:, 0:TPB, 0:3],
                                        pp[:, TPB:32, 0:3], ALU.subtract)
                sq = st.tile([128, TPB, 3], f32, name="sq", tag="sq")
                nc.vector.tensor_tensor(sq[:], ruu_t[b][:, :, 0:3],
                                        ruu_t[b][:, :, 0:3], ALU.mult)
                nc.vector.reduce_sum(d2_all[:, bsl], sq[:], axis=mybir.AxisListType.X)
                nc.scalar.activation(d_all[:, bsl], d2_all[:, bsl], AF.Sqrt)
                nc.vector.reciprocal(invd_all[:, bsl], d_all[:, bsl])
                invb3 = invd_all[:, bsl].unsqueeze(2).broadcast_to([128, TPB, 3])
                nc.vector.tensor_tensor(ruu_t[b][:, :, 0:3], ruu_t[b][:, :, 0:3],
                                        invb3, ALU.mult)
                o = 3
                for a_ in range(3):
                    n_ = 3 - a_
                    nc.vector.tensor_tensor(
                        ruu_t[b][:, :, o:o + n_],
                        ruu_t[b][:, :, a_:a_ + 1].broadcast_to([128, TPB, n_]),
                        ruu_t[b][:, :, a_:3], ALU.mult)
                    o += n_
                args = st.tile([128, TPB, C], f32, name="args", tag="args", bufs=1)
                nc.vector.tensor_tensor(
                    args[:],
                    npi[:].unsqueeze(1).broadcast_to([128, TPB, C]),
                    d_all[:, bsl].unsqueeze(2).broadcast_to([128, TPB, C]), ALU.mult)
                tk = st.tile([128, TPB, C], f32, name="tk", tag="tk", bufs=1)
                nc.vector.tensor_scalar(tk[:], args[:], INV2PI, MAGIC,
                                        op0=ALU.mult, op1=ALU.add)
                nc.vector.tensor_scalar(tk[:], tk[:], MAGIC, None, op0=ALU.subtract)
                red = st.tile([128, TPB, C], f32, name="red", tag="red", bufs=1)
                nc.vector.cody_waite_cascade(
                    red[:].rearrange("p t c -> p (t c)"),
                    args[:].rearrange("p t c -> p (t c)"),
                    tk[:].rearrange("p t c -> p (t c)"),
                    CW1, CW2, CW3)
                sn = st.tile([128, TPB, C], f32, name="sn", tag="sn", bufs=1)
                nc.scalar.activation(sn[:], red[:], AF.Sin)
                nc.vector.tensor_tensor(
                    rad_t[b][:, :, :], sn[:],
                    invd_all[:, bsl].unsqueeze(2).broadcast_to([128, TPB, C]),
                    ALU.mult)

            # ---- layers ----
            for l in range(L):
                for b in range(B):
                    psum_blk = psA.tile([128, F160], f32, name="psum_blk", tag="pa")
                    if l > 0:
                        hgb = st.tile([128, TPB, FPAD], f32, name="hgb", tag="hgb",
                                      bufs=2)
                        for qk in range(2):
                            nc.gpsimd.dma_gather(
                                hgb[:, 8 * qk:8 * (qk + 1), :], h_full[:, :],
                                slotx[:, b * 128 + 64 * qk:b * 128 + 64 * (qk + 1)],
                                num_idxs=1024, num_idxs_reg=1024,
                                elem_size=FPAD)
                    for ck in range(NCHUNK):
                        csl = slice(ck * CH, (ck + 1) * CH)
                        msg = st3.tile([128, CH, F160], f32, name="msg",
                                       tag="msg", bufs=2)
                        radc = rad_t[b][:, csl, :]
                        ruuc = ruu_t[b][:, csl, :]
                        c12 = msg[:, :, 16:160].rearrange("p t (s c) -> p t s c", c=C)
                        dual = False
                        if l == 0:
                            nc.vector.tensor_copy(msg[:, :, 0:16], radc)
                        else:
                            hg = hgb[:, csl, :]
                            nc.vector.tensor_tensor(msg[:, :, 0:16], radc,
                                                    hg[:, :, 0:16], ALU.mult)
                        q_b9 = msg[:, :, 0:16].unsqueeze(2).broadcast_to(
                            [128, CH, 9, C])
                        ruu_bc = ruuc.unsqueeze(3).broadcast_to([128, CH, 9, C])
                        if l == 0:
                            nc.vector.tensor_tensor(c12, q_b9, ruu_bc, ALU.mult)
                        else:
                            rad_b9 = radc.unsqueeze(2).broadcast_to([128, CH, 9, C])
                            hgc = hg[:, :, 16:160].rearrange(
                                "p t (s c) -> p t s c", c=C)
                            tmp = st.tile([128, CH, 9, C], f32, name="tmp",
                                          tag="tmp", bufs=2)
                            dual = (b * NCHUNK + ck) % 2 == 0
                            nc.vector.tensor_tensor(c12, q_b9, ruu_bc, ALU.mult)
                            nc.gpsimd.tensor_tensor(tmp[:], hgc, rad_b9, ALU.mult)
                            if not dual:
                                nc.vector.tensor_tensor(c12, c12, tmp[:], ALU.add)
                        oh = st.tile([128, CH, 128], f32, name="oh", tag="oh", bufs=3)
                        nc.vector.tensor_tensor(
                            oh[:],
                            iota_f[:].unsqueeze(1).broadcast_to([128, CH, 128]),
                            rv_all[:, b * TPB + ck * CH: b * TPB + (ck + 1) * CH]
                                .unsqueeze(2).broadcast_to([128, CH, 128]),
                            ALU.is_equal)
                        last = (ck == NCHUNK - 1)
                        for t in range(CH):
                            nc.tensor.matmul(psum_blk[:], oh[:, t, :], msg[:, t, :],
                                             start=(ck == 0 and t == 0),
                                             stop=(last and not (l > 0 and dual)
                                                   and t == CH - 1),
                                             skip_group_check=True)
                        if l > 0 and dual:
                            t2f = tmp[:].rearrange("p t s c -> p t (s c)")
                            for t in range(CH):
                                nc.tensor.matmul(psum_blk[:, 16:160], oh[:, t, :],
                                                 t2f[:, t, :],
                                                 start=False,
                                                 stop=(last and t == CH - 1),
                                                 skip_group_check=True)
                    absb = st.tile([128, F160], f32, name="absb", tag="absb")
                    nc.scalar.copy(absb[:], psum_blk[:])
                    bcols = slice(b * 128, (b + 1) * 128)
                    for g in range(5):
                        ptg = psT.tile([32, 128], f32, name="ptg", tag="pt")
                        nc.tensor.transpose(ptg[:], absb[:, 32 * g:32 * (g + 1)],
                                            ident[:])
                        nc.scalar.copy(ATg[g][:, bcols], ptg[:])

                NCHK = (W + 511) // 512
                for cc in range(NCHK):
                    cs = slice(cc * 512, min((cc + 1) * 512, W))
                    ncols = cs.stop - cs.start
                    pw = psW.tile([128, 512], f32, name="pw", tag="pw", bufs=2)
                    pwb = psW.tile([32, 512], f32, name="pwb", tag="pwb", bufs=1)
                    prep_ = psW.tile([128, 512], f32, name="prep_", tag="prep_",
                                     bufs=1)
                    for g in range(4):
                        nc.tensor.matmul(
                            pw[32 * g:32 * (g + 1), :ncols],
                            abw_t[:, (l * 5 + g) * 32:(l * 5 + g + 1) * 32],
                            ATg[g][:, cs], tile_position=(0, 32 * g))
                    nc.tensor.matmul(pwb[:, :ncols],
                                     abw_t[:, (l * 5 + 4) * 32:(l * 5 + 5) * 32],
                                     ATg[4][:, cs])
                    a0sb = st.tile([16, 512], f32, name="a0sb", tag="a0sb")
                    nc.scalar.copy(a0sb[:, :ncols], pw[0:16, :ncols])
                    nc.tensor.matmul(prep_[:, :ncols], rep_t[:], a0sb[:, :ncols])
                    nc.vector.tensor_scalar(Fa[:, cs], prep_[:, :ncols],
                                            wsva[:, l * 5 + 1:l * 5 + 2],
                                            wsva[:, l * 5 + 0:l * 5 + 1],
                                            op0=ALU.mult, op1=ALU.add)
                    nc.vector.tensor_scalar(Fb[:, cs], prep_[0:32, :ncols],
                                            wsvb[:, l * 5 + 1:l * 5 + 2],
                                            wsvb[:, l * 5 + 0:l * 5 + 1],
                                            op0=ALU.mult, op1=ALU.add)
                    nc.vector.tensor_tensor(Fa[:, cs], Fa[:, cs], pw[:, :ncols],
                                            ALU.mult)
                    nc.vector.tensor_tensor(Fb[:, cs], Fb[:, cs], pwb[:, :ncols],
                                            ALU.mult)
                if l == 0:
                    nc.vector.tensor_scalar(hTa[:], Fa[:], wsva[:, 3:4],
                                            wsva[:, 4:5], op0=ALU.mult, op1=ALU.add)
                    nc.vector.tensor_scalar(hTb[:], Fb[:], wsvb[:, 3:4],
                                            wsvb[:, 4:5], op0=ALU.mult, op1=ALU.add)
                else:
                    nc.vector.tensor_scalar(hTa[:], hTa[:],
                                            wsva[:, l * 5 + 2:l * 5 + 3],
                                            None, op0=ALU.mult)
                    nc.vector.scalar_tensor_tensor(hTa[:], Fa[:],
                                                   wsva[:, l * 5 + 3:l * 5 + 4],
                                                   hTa[:], ALU.mult, ALU.add)
                    nc.vector.tensor_scalar(hTb[:], hTb[:],
                                            wsvb[:, l * 5 + 2:l * 5 + 3],
                                            None, op0=ALU.mult)
                    nc.vector.scalar_tensor_tensor(hTb[:], Fb[:],
                                                   wsvb[:, l * 5 + 3:l * 5 + 4],
                                                   hTb[:], ALU.mult, ALU.add)

                if l == 0:
                    for b in range(B):
                        bcols = slice(b * 128, (b + 1) * 128)
                        pta = psT.tile([128, 128], f32, name="pta", tag="pt")
                        nc.tensor.transpose(pta[:], hTa[:, bcols], ident[:])
                        ptb2 = psT.tile([128, 32], f32, name="ptb2", tag="pt")
                        nc.tensor.transpose(ptb2[:], hTb[:, bcols],
                                            ident[0:32, 0:32])
                        hsb = st.tile([128, FPAD], f32, name="hsb", tag="hsb")
                        nc.scalar.copy(hsb[:, 0:128], pta[:])
                        nc.scalar.copy(hsb[:, 128:160], ptb2[:])
                        nc.gpsimd.memset(hsb[:, 160:192], 0.0)
                        nc.sync.dma_start(h_local[b * 128:(b + 1) * 128, :], hsb[:])
                    nc.gpsimd.collective_compute(
                        "AllGather", ALU.bypass,
                        replica_groups=[list(range(n_cores))],
                        ins=[h_local.opt()], outs=[h_full.opt()])

            # ---- pooling + head ----
            psum_pool = psA.tile([64, F160], f32, name="psum_pool", tag="pa")
            for b in range(B):
                bcols = slice(b * 128, (b + 1) * 128)
                pta = psT.tile([128, 128], f32, name="pta", tag="pt")
                nc.tensor.transpose(pta[:], hTa[:, bcols], ident[:])
                ptb2 = psT.tile([128, 32], f32, name="ptb2", tag="pt")
                nc.tensor.transpose(ptb2[:], hTb[:, bcols], ident[0:32, 0:32])
                flat = st.tile([128, F160], f32, name="flat", tag="flat")
                nc.scalar.copy(flat[:, 0:128], pta[:])
                nc.scalar.copy(flat[:, 128:160], ptb2[:])
                goh = st.tile([128, 64], f32, name="goh", tag="goh")
                nc.vector.tensor_scalar(goh[:], iota_f[:, 0:64],
                                        batch_t[:, b:b + 1], None, op0=ALU.is_equal)
                nc.tensor.matmul(psum_pool[:], goh[:], flat[:],
                                 start=(b == 0), stop=(b == B - 1))
            pooled = st.tile([64, F160], f32, name="pooled", tag="pooled")
            nc.scalar.copy(pooled[:], psum_pool[:])
            nc.sync.dma_start(ar_in[:], pooled[:])
            nc.gpsimd.collective_compute(
                "AllReduce", ALU.add,
                replica_groups=[list(range(n_cores))],
                ins=[ar_in.opt()], outs=[ar_out.opt()])
            pooled_ar = st.tile([64, F160], f32, name="pooled_ar", tag="pooled_ar")
            nc.sync.dma_start(pooled_ar[:], ar_out[:])
            pt1 = psT.tile([128, 128], f32, name="pta", tag="pt")
            nc.tensor.transpose(pt1[:, 0:64], pooled_ar[:, 0:128], ident[0:64, 0:64])
            pt2 = psT.tile([32, 64], f32, name="pt2", tag="pt")
            nc.tensor.transpose(pt2[:], pooled_ar[:, 128:160], ident[0:64, 0:64])
            pt1s = st.tile([128, 64], f32, name="pt1s", tag="pt1s")
            nc.scalar.copy(pt1s[:], pt1[:, 0:64])
            pt2s = st.tile([32, 64], f32, name="pt2s", tag="pt2s")
            nc.scalar.copy(pt2s[:], pt2[:])
            pso = psA.tile([64, 2], f32, name="pso", tag="pa")
            nc.tensor.matmul(pso[:], pt1s[:], pw1[:], start=True, stop=False)
            nc.tensor.matmul(pso[:], pt2s[:], pw2[:], start=False, stop=True)
            outsb = st.tile([64, 2], f32, name="outsb", tag="outsb")
            nc.vector.tensor_tensor(outsb[:], pso[:], pb[:], ALU.add)
            nc.sync.dma_start(out_t[:], outsb[:])

    nc.compile()
    return nc


def _run_spmd(nc, in_maps, n_cores):
    import jax
    from jax.sharding import Mesh, PartitionSpec
    from jax.experimental.shard_map import shard_map
    from concourse.bass2jax import (_bass_exec_p, install_neuronx_cc_hook,
                                    partition_id_tensor)
    install_neuronx_cc_hook()
    partition_name = nc.partition_id_tensor.name if nc.partition_id_tensor else None
    in_names, out_names, out_avals, zero_outs = [], [], [], []
    for alloc in nc.m.functions[0].allocations:
        if not isinstance(alloc, mybir.MemoryLocationSet):
            continue
        name = alloc.memorylocations[0].name
        if alloc.kind == "ExternalInput":
            if name != partition_name:
                in_names.append(name)
        elif alloc.kind == "ExternalOutput":
            shape, dt = alloc.tensor_shape, mybir.dt.np(alloc.dtype)
            out_names.append(name)
            out_avals.append(jax.core.ShapedArray(shape, dt))
            zero_outs.append(np.zeros(shape, dt))
    all_in_names = in_names + out_names + ([partition_name] if partition_name else [])

    def _body(*args):
        operands = list(args)
        if partition_name is not None:
            operands.append(partition_id_tensor())
        outs = _bass_exec_p.bind(
            *operands, out_avals=tuple(out_avals), in_names=tuple(all_in_names),
            out_names=tuple(out_names), lowering_input_output_aliases=(),
            sim_require_finite=False, sim_require_nnan=False, nc=nc)
        return tuple(outs)

    devices = jax.devices()[:n_cores]
    mesh = Mesh(np.asarray(devices), ("core",))
    n_params, n_outs = len(in_names), len(out_names)
    fn = jax.jit(
        shard_map(_body, mesh=mesh,
                  in_specs=(PartitionSpec("core"),) * (n_params + n_outs),
                  out_specs=(PartitionSpec("core"),) * n_outs, check_rep=False),
        keep_unused=True)
    concat_in = [
        np.concatenate([np.asarray(in_maps[c][nm]) for c in range(n_cores)], axis=0)
        for nm in in_names]
    concat_zero = [np.zeros((n_cores * z.shape[0], *z.shape[1:]), z.dtype)
                   for z in zero_outs]
    outs = fn(*concat_in, *concat_zero)
    res = {}
    for i, nm in enumerate(out_names):
        res[nm] = np.asarray(outs[i]).reshape(n_cores, *out_avals[i].shape)[0]
    return res


def kernel(pos, emb_weight, ab_w, ws_w, channel_w, message_w, pred_W, pred_b,
           atoms, edge_index, batch):
    pos = np.asarray(pos, dtype=np.float32)
    emb_weight = np.asarray(emb_weight, dtype=np.float32)
    ab_w = np.asarray(ab_w, dtype=np.float32)
    ws_w = np.asarray(ws_w, dtype=np.float32)
    channel_w = np.asarray(channel_w, dtype=np.float32)
    message_w = np.asarray(message_w, dtype=np.float32)
    pred_W = np.asarray(pred_W, dtype=np.float32)
    pred_b = np.asarray(pred_b, dtype=np.float32)
    edge_index = np.asarray(edge_index)
    batch = np.asarray(batch)

    prep = _host_prep(pos, emb_weight, ab_w, ws_w, channel_w, message_w,
                      pred_W, pred_b, atoms, edge_index, batch)
    nc = _build(prep)
    B = prep["B"]
    in_maps = []
    for k in range(NCORES):
        in_maps.append({
            "pos_pad": prep["pos_pad"],
            "npi": prep["npi"],
            "pos_idx": prep["pos_idx"][k],
            "slot_idxb": prep["slot_idxb"][k],
            "rv_dev": np.ascontiguousarray(prep["rv_dev"][k].reshape(128, B * TPB)),
            "batch_slot": prep["batch_slot"][k],
            "abw_dev": prep["abw_dev"],
            "rep16": prep["rep16"],
            "wsv_a": prep["wsv_a"],
            "wsv_b": prep["wsv_b"],
            "predW1": prep["predW1"],
            "predW2": prep["predW2"],
            "predb": prep["predb_rep"],
        })
    res = _run_spmd(nc, in_maps, NCORES)
    return res["out"].astype(np.float32)



# revision 6
# speedup vs baseline: 2.0064x; 2.0064x over previous
"""CartesianMACE message-passing kernel for 8 Trainium2 NeuronCores.

Self-contained: kernel(**inputs) takes the FULL inputs and returns the FULL
[G, OUT] output. Edges are sharded by receiver across 8 cores
(receiver-contiguous blocks of <=128 nodes / <=2048 edges); a Bass/Tile SPMD
kernel does segment-sum via one-hot matmuls into PSUM, node-level channel
mixing on the PE, an AllGather of node features between the two layers, and
per-graph pooling + AllReduce + prediction head on-device.

The compute core runs in bf16 (messages, one-hots, matmuls, channel mixing)
with fp32 geometry (distances, radial embedding) and fp32 accumulation in
PSUM; host-side work is index/layout preparation only.
"""
import sys
sys.path.insert(0, "/opt/trn_rl_repo")
import numpy as np
import concourse.bass as bass
import concourse.bacc as bacc
import concourse.mybir as mybir
from concourse import tile, masks

f32 = mybir.dt.float32
bf16 = mybir.dt.bfloat16
i16 = mybir.dt.int16
i32 = mybir.dt.int32
AF = mybir.ActivationFunctionType
ALU = mybir.AluOpType

PI = float(np.pi)
TWO_PI = float(2 * np.pi)
MAGIC = 12582912.0  # 1.5*2^23: add/sub rounds fp32 to nearest int
CW1 = float(np.float32(6.28125))
CW2 = float(np.float32(TWO_PI - CW1))
CW3 = float(TWO_PI - CW1 - float(np.float32(TWO_PI - CW1)))
INV2PI = float(1.0 / TWO_PI)

NCORES = 8
C = 16
L = 2
RANKS = 3
EPB = 2048
TPB = 16
BLKN = 128
F160 = 160
FPAD = 256  # bf16 h row: 512B, gather-friendly
POSPAD = 64
KMAP = [0, 1, 1, 1, 2, 2, 2, 2, 2, 2]
SYM_A = np.array([0, 0, 0, 1, 1, 2])
SYM_B = np.array([0, 1, 2, 1, 2, 2])


def _wrap_idx(idx, cols):
    n = idx.shape[0]
    w = np.zeros((16, cols), dtype=np.int16)
    w[np.arange(n) % 16, np.arange(n) // 16] = idx.astype(np.int16)
    return np.tile(w, (8, 1))


def _host_prep(pos, emb_weight, ab_w, ws_w, channel_w, message_w, pred_W, pred_b,
               atoms, edge_index, batch):
    N = pos.shape[0]
    OUT = pred_W.shape[1]
    src = np.asarray(edge_index[0], dtype=np.int64)
    rcv = np.asarray(edge_index[1], dtype=np.int64)

    deg = np.bincount(rcv, minlength=N)
    assert deg.max() <= EPB

    blocks = []
    n0, cur = 0, 0
    for n in range(N):
        d = deg[n]
        if n > n0 and (n - n0 >= BLKN or cur + d > EPB):
            blocks.append((n0, n))
            n0, cur = n, 0
        cur += d
    blocks.append((n0, N))
    B = (len(blocks) + NCORES - 1) // NCORES
    while len(blocks) < NCORES * B:
        blocks.append((N, N))
    NSLOT = NCORES * B * BLKN
    assert NSLOT < 32768  # int16 gather indices

    node_slot = np.full(N, -1, dtype=np.int64)
    for bi, (a, b) in enumerate(blocks):
        node_slot[a:b] = bi * BLKN + np.arange(b - a)

    order = np.argsort(rcv, kind="stable")
    rs, ss = rcv[order], src[order]
    starts = np.searchsorted(rs, [a for a, _ in blocks])
    ends = np.searchsorted(rs, [b for _, b in blocks])

    esnd = np.zeros((NCORES, B, EPB), dtype=np.int64)
    ercv = np.ones((NCORES, B, EPB), dtype=np.int64)
    eslot = np.zeros((NCORES, B, EPB), dtype=np.int64)
    rv_adj = np.full((NCORES, B, EPB), -1.0, dtype=np.float32)
    for bi, (a, b) in enumerate(blocks):
        k, bb = divmod(bi, B)
        e0, e1 = starts[bi], ends[bi]
        ne = e1 - e0
        esnd[k, bb, :ne] = ss[e0:e1]
        ercv[k, bb, :ne] = rs[e0:e1]
        eslot[k, bb, :ne] = node_slot[ss[e0:e1]]
        rv_adj[k, bb, :ne] = (rs[e0:e1] - a).astype(np.float32)

    rv_dev = np.transpose(rv_adj.reshape(NCORES, B, TPB, 128), (0, 3, 1, 2)).copy()

    pos_idx = np.zeros((NCORES, B * 128, 256), dtype=np.int16)
    slot_idxb = np.zeros((NCORES, 128, B * 128), dtype=np.int16)
    for k in range(NCORES):
        for bb in range(B):
            tok = np.concatenate([esnd[k, bb], ercv[k, bb]])
            pos_idx[k, bb * 128:(bb + 1) * 128, :] = _wrap_idx(tok, 256)
            slot_idxb[k, :, bb * 128:(bb + 1) * 128] = _wrap_idx(eslot[k, bb], 128)

    batch_slot = np.full((NCORES, 128, B), -1.0, dtype=np.float32)
    for bi, (a, b) in enumerate(blocks):
        k, bb = divmod(bi, B)
        batch_slot[k, :b - a, bb] = batch[a:b].astype(np.float32)

    pos_pad = np.zeros((N, POSPAD), dtype=np.float32)
    pos_pad[:, :3] = pos
    npi = np.tile((np.arange(1, C + 1) * PI).astype(np.float32)[None, :], (128, 1))

    emb0 = np.asarray(emb_weight[0], dtype=np.float32)
    abwT = np.zeros((L, RANKS, C, C), dtype=np.float32)
    for c in range(RANKS):
        abwT[0, c] = (ab_w[0, c] * emb0[None, :]).T
        abwT[1, c] = (ab_w[1, c] * (2.0 if c == 0 else 1.0)).T
    # block-diagonal packing: feats 0:128 = groups 0..7, feats 128:160 = 8,9
    abwD = np.zeros((L, 128, 128), dtype=np.float32)
    abwB = np.zeros((L, 32, 32), dtype=np.float32)
    for l in range(L):
        for g in range(8):
            abwD[l, g * 16:(g + 1) * 16, g * 16:(g + 1) * 16] = abwT[l, KMAP[g]]
        for g in range(2):
            abwB[l, g * 16:(g + 1) * 16, g * 16:(g + 1) * 16] = abwT[l, KMAP[8 + g]]
    abwD_dev = np.transpose(abwD, (1, 0, 2)).reshape(128, L * 128)
    abwB_dev = np.transpose(abwB, (1, 0, 2)).reshape(32, L * 32)
    rep16 = np.zeros((16, 128), dtype=np.float32)
    rep16[np.arange(128) % 16, np.arange(128)] = 1.0

    def vec_for(l, fn):
        v = np.zeros(160, dtype=np.float32)
        for k in range(10):
            v[k * 16:(k + 1) * 16] = fn(l, KMAP[k])
        return v
    wsv = np.zeros((L, 160, 5), dtype=np.float32)
    for l in range(L):
        wsv[l, :, 0] = vec_for(l, lambda l_, c: ws_w[l_, 0, c])
        wsv[l, :, 1] = vec_for(l, lambda l_, c: ws_w[l_, 1, c])
        wsv[l, :, 2] = vec_for(l, lambda l_, c: channel_w[l_, c].sum(0))
        wsv[l, :, 3] = vec_for(l, lambda l_, c: message_w[l_, c].sum(0))
    wsv[0, 0:16, 4] = emb0 * channel_w[0, 0].sum(0)
    wsv_a = np.transpose(wsv[:, 0:128, :], (1, 0, 2)).reshape(128, L * 5).copy()
    wsv_b = np.transpose(wsv[:, 128:160, :], (1, 0, 2)).reshape(32, L * 5).copy()

    predW_eff = np.zeros((160, OUT), dtype=np.float32)
    predW_eff[0:16] = pred_W[0:16]
    for x in range(3):
        for ch in range(C):
            predW_eff[16 + x * 16 + ch] = pred_W[16 + ch * 3 + x]
    for s in range(6):
        a_, b_ = SYM_A[s], SYM_B[s]
        for ch in range(C):
            w = pred_W[64 + ch * 9 + a_ * 3 + b_].copy()
            if a_ != b_:
                w = w + pred_W[64 + ch * 9 + b_ * 3 + a_]
            predW_eff[64 + s * 16 + ch] = w
    predb_rep = np.tile(np.asarray(pred_b, dtype=np.float32)[None, :], (64, 1))

    def as_bf(x):
        import ml_dtypes
        return np.asarray(x, dtype=ml_dtypes.bfloat16)

    return dict(N=N, B=B, NSLOT=NSLOT, pos_idx=pos_idx, slot_idxb=slot_idxb,
                rv_dev=rv_dev, batch_slot=batch_slot, pos_pad=pos_pad, npi=npi,
                abwD=as_bf(abwD_dev), abwB=as_bf(abwB_dev), rep16=as_bf(rep16),
                wsv_a=wsv_a, wsv_b=wsv_b,
                predW1=predW_eff[0:128].copy(), predW2=predW_eff[128:160].copy(),
                predb_rep=predb_rep)


def _build(prep, n_cores=NCORES, sim_mode=False):
    B = prep["B"]
    NSLOT = prep["NSLOT"]
    W = B * 128

    nc = bacc.Bacc("TRN2", target_bir_lowering=False, debug=False,
                   num_devices=1 if sim_mode else n_cores)

    pos_pad = nc.dram_tensor("pos_pad", [prep["N"], POSPAD], f32, kind="ExternalInput")
    npi_in = nc.dram_tensor("npi", [128, C], f32, kind="ExternalInput")
    pidx_in = nc.dram_tensor("pos_idx", [B * 128, 256], i16, kind="ExternalInput")
    slot_in = nc.dram_tensor("slot_idxb", [128, B * 128], i16, kind="ExternalInput")
    rv_in = nc.dram_tensor("rv_dev", [128, B * TPB], f32, kind="ExternalInput")
    batch_in = nc.dram_tensor("batch_slot", [128, B], f32, kind="ExternalInput")
    abwD_in = nc.dram_tensor("abwD", [128, L * 128], bf16, kind="ExternalInput")
    abwB_in = nc.dram_tensor("abwB", [32, L * 32], bf16, kind="ExternalInput")
    rep_in = nc.dram_tensor("rep16", [16, 128], bf16, kind="ExternalInput")
    wsva_in = nc.dram_tensor("wsv_a", [128, L * 5], f32, kind="ExternalInput")
    wsvb_in = nc.dram_tensor("wsv_b", [32, L * 5], f32, kind="ExternalInput")
    pw1_in = nc.dram_tensor("predW1", [128, 2], f32, kind="ExternalInput")
    pw2_in = nc.dram_tensor("predW2", [32, 2], f32, kind="ExternalInput")
    pb_in = nc.dram_tensor("predb", [64, 2], f32, kind="ExternalInput")
    out_t = nc.dram_tensor("out", [64, 2], f32, kind="ExternalOutput")

    with tile.TileContext(nc) as tc:
        with (
            tc.tile_pool(name="const", bufs=1) as cpool,
            tc.tile_pool(name="geo", bufs=1) as geo,
            tc.tile_pool(name="st", bufs=2) as st,
            tc.tile_pool(name="wide", bufs=1) as wd,
            tc.tile_pool(name="psA", bufs=2, space="PSUM") as psA,
            tc.tile_pool(name="psT", bufs=2, space="PSUM") as psT,
            tc.tile_pool(name="psW", bufs=1, space="PSUM") as psW,
            tc.tile_pool(name="dram", bufs=1, space="DRAM") as dram,
        ):
            def ct(shape, name, dt=f32):
                return cpool.tile(shape, dt, name=name, tag=name)

            npi = ct([128, C], "npi_t")
            nc.sync.dma_start(npi[:], npi_in[:])
            rv_all = ct([128, B * TPB], "rv_all")
            nc.sync.dma_start(rv_all[:], rv_in[:])
            batch_t = ct([128, B], "batch_t")
            nc.sync.dma_start(batch_t[:], batch_in[:])
            abwD_t = ct([128, L * 128], "abwD_t", bf16)
            nc.sync.dma_start(abwD_t[:], abwD_in[:])
            abwB_t = ct([32, L * 32], "abwB_t", bf16)
            nc.sync.dma_start(abwB_t[:], abwB_in[:])
            rep_t = ct([16, 128], "rep_t", bf16)
            nc.sync.dma_start(rep_t[:], rep_in[:])
            wsva = ct([128, L * 5], "wsva_t")
            nc.sync.dma_start(wsva[:], wsva_in[:])
            wsvb = ct([32, L * 5], "wsvb_t")
            nc.sync.dma_start(wsvb[:], wsvb_in[:])
            pw1 = ct([128, 2], "pw1_t")
            nc.sync.dma_start(pw1[:], pw1_in[:])
            pw2 = ct([32, 2], "pw2_t")
            nc.sync.dma_start(pw2[:], pw2_in[:])
            pb = ct([64, 2], "pb_t")
            nc.sync.dma_start(pb[:], pb_in[:])
            iota_i = ct([128, 128], "iota_i", i32)
            nc.gpsimd.iota(iota_i[:], pattern=[[1, 128]], base=0, channel_multiplier=0)
            iota_bf = ct([128, 128], "iota_bf", bf16)
            nc.vector.tensor_copy(iota_bf[:], iota_i[:])
            identb = ct([128, 128], "identb", bf16)
            masks.make_identity(nc, identb[:])
            identf = ct([64, 64], "identf")
            masks.make_identity(nc, identf[:])

            # per-block persistent tiles
            mblk = [geo.tile([128, TPB, F160], bf16, name=f"mblk{b}", tag=f"mblk{b}")
                    for b in range(B)]
            relb = [geo.tile([128, TPB, 3], f32, name=f"relb{b}", tag=f"relb{b}")
                    for b in range(B)]
            d2_all = geo.tile([128, B * TPB], f32, name="d2_all", tag="d2_all")
            d_all = geo.tile([128, B * TPB], f32, name="d_all", tag="d_all")
            invd_all = geo.tile([128, B * TPB], f32, name="invd_all", tag="invd_all")

            def wt(shape, name, dt=bf16):
                return wd.tile(shape, dt, name=name, tag=name)
            AT1 = wt([128, W], "AT1")
            AT2 = wt([32, W], "AT2")
            Fa = AT1   # dead after the pw matmul of each chunk; reuse in place
            Fb = AT2
            hTa = wt([128, W], "hTa")
            hTb = wt([32, W], "hTb")

            h_local = dram.tile([B * 128, FPAD], bf16, name="h_local", tag="h_local")
            if sim_mode:
                h_full = dram.tile([NSLOT, FPAD], bf16, name="h_full", tag="h_full")
            else:
                h_full = dram.tile([NSLOT, FPAD], bf16, name="h_full", tag="h_full",
                                   addr_space="Shared")
            ar_in = dram.tile([64, F160], f32, name="ar_in", tag="ar_in")
            ar_out = dram.tile([64, F160], f32, name="ar_out", tag="ar_out")

            # ---- geometry: pipelined in groups (sqrt batched per group) ----
            GRP = 5
            for g0 in range(0, B, GRP):
                gblocks = range(g0, min(g0 + GRP, B))
                for b in gblocks:
                    pidx = st.tile([128, 256], i16, name="pidx", tag="pidx")
                    nc.sync.dma_start(pidx[:], pidx_in[b * 128:(b + 1) * 128, :])
                    pp = st.tile([128, 32, POSPAD], f32, name="pp", tag="pp", bufs=2)
                    for qk in range(4):
                        nc.gpsimd.dma_gather(pp[:, 8 * qk:8 * (qk + 1), :],
                                             pos_pad[:, :],
                                             pidx[:, 64 * qk:64 * (qk + 1)],
                                             num_idxs=1024, num_idxs_reg=1024,
                                             elem_size=POSPAD)
                    bsl = slice(b * TPB, (b + 1) * TPB)
                    nc.vector.tensor_tensor(relb[b][:], pp[:, 0:TPB, 0:3],
                                            pp[:, TPB:32, 0:3], ALU.subtract)
                    sq = st.tile([128, TPB, 3], f32, name="sq", tag="sq")
                    nc.vector.tensor_tensor(sq[:], relb[b][:], relb[b][:], ALU.mult)
                    nc.vector.reduce_sum(d2_all[:, bsl], sq[:],
                                         axis=mybir.AxisListType.X)
                gsl = slice(g0 * TPB, min(g0 + GRP, B) * TPB)
                nc.scalar.sqrt(d_all[:, gsl], d2_all[:, gsl])
                nc.vector.reciprocal(invd_all[:, gsl], d_all[:, gsl])
                for b in gblocks:
                    bsl = slice(b * TPB, (b + 1) * TPB)
                    invb3 = invd_all[:, bsl].unsqueeze(2).broadcast_to([128, TPB, 3])
                    nc.vector.tensor_tensor(relb[b][:], relb[b][:], invb3, ALU.mult)
                    ruu9 = st.tile([128, TPB, 9], f32, name="ruu9", tag="ruu9",
                                   bufs=2)
                    nc.vector.tensor_copy(ruu9[:, :, 0:3], relb[b][:])
                    o = 3
                    for a_ in range(3):
                        n_ = 3 - a_
                        nc.vector.tensor_tensor(
                            ruu9[:, :, o:o + n_],
                            relb[b][:, :, a_:a_ + 1].broadcast_to([128, TPB, n_]),
                            relb[b][:, :, a_:3], ALU.mult)
                        o += n_
                    args = st.tile([128, TPB, C], f32, name="args", tag="args",
                                   bufs=1)
                    nc.vector.tensor_tensor(
                        args[:],
                        npi[:].unsqueeze(1).broadcast_to([128, TPB, C]),
                        d_all[:, bsl].unsqueeze(2).broadcast_to([128, TPB, C]),
                        ALU.mult)
                    tk = st.tile([128, TPB, C], f32, name="tk", tag="tk", bufs=1)
                    nc.vector.tensor_scalar(tk[:], args[:], INV2PI, MAGIC,
                                            op0=ALU.mult, op1=ALU.add)
                    nc.vector.tensor_scalar(tk[:], tk[:], MAGIC, None,
                                            op0=ALU.subtract)
                    red = st.tile([128, TPB, C], f32, name="red", tag="red", bufs=1)
                    nc.vector.cody_waite_cascade(
                        red[:].rearrange("p t c -> p (t c)"),
                        args[:].rearrange("p t c -> p (t c)"),
                        tk[:].rearrange("p t c -> p (t c)"),
                        CW1, CW2, CW3)
                    sn = st.tile([128, TPB, C], f32, name="sn", tag="sn", bufs=2)
                    nc.scalar.activation(sn[:], red[:], AF.Sin)
                    nc.vector.tensor_tensor(
                        mblk[b][:, :, 0:16], sn[:],
                        invd_all[:, bsl].unsqueeze(2).broadcast_to([128, TPB, C]),
                        ALU.mult)
                    nc.vector.tensor_tensor(
                        mblk[b][:, :, 16:160].rearrange("p t (s c) -> p t s c", c=C),
                        ruu9[:].unsqueeze(3).broadcast_to([128, TPB, 9, C]),
                        mblk[b][:, :, 0:16].unsqueeze(2).broadcast_to(
                            [128, TPB, 9, C]),
                        ALU.mult)

            # ---- layers ----
            for l in range(L):
                for b in range(B):
                    if l > 0:
                        slot_pb = st.tile([128, 128], i16, name="slot_pb",
                                          tag="slot_pb", bufs=2)
                        nc.sync.dma_start(slot_pb[:],
                                          slot_in[:, b * 128:(b + 1) * 128])
                        hgb = st.tile([128, TPB, FPAD], bf16, name="hgb", tag="hgb",
                                      bufs=2)
                        for qk in range(2):
                            nc.gpsimd.dma_gather(
                                hgb[:, 8 * qk:8 * (qk + 1), :], h_full[:, :],
                                slot_pb[:, 64 * qk:64 * (qk + 1)],
                                num_idxs=1024, num_idxs_reg=1024,
                                elem_size=FPAD)
                        msg = st.tile([128, TPB, F160], bf16, name="msg",
                                      tag="msg", bufs=2)
                        nc.vector.tensor_tensor(
                            msg[:].rearrange("p t (s c) -> p t s c", c=C),
                            mblk[b][:].rearrange("p t (s c) -> p t s c", c=C),
                            hgb[:, :, 0:16].unsqueeze(2).broadcast_to(
                                [128, TPB, 10, C]),
                            ALU.mult)
                        tmp = st.tile([128, TPB, 144], bf16, name="tmp",
                                      tag="tmp", bufs=2)
                        nc.vector.tensor_tensor(
                            tmp[:].rearrange("p t (s c) -> p t s c", c=C),
                            hgb[:, :, 16:160].rearrange("p t (s c) -> p t s c", c=C),
                            mblk[b][:, :, 0:16].unsqueeze(2).broadcast_to(
                                [128, TPB, 9, C]),
                            ALU.mult)
                        nc.vector.tensor_tensor(msg[:, :, 16:160], msg[:, :, 16:160],
                                                tmp[:], ALU.add)
                    else:
                        msg = mblk[b]
                    oh = st.tile([128, TPB, 128], bf16, name="oh", tag="oh", bufs=2)
                    for t in range(TPB):
                        nc.vector.tensor_scalar(
                            oh[:, t, :], iota_bf[:],
                            rv_all[:, b * TPB + t:b * TPB + t + 1], None,
                            op0=ALU.is_equal)
                    psum_blk = psA.tile([128, F160], f32, name="psum_blk", tag="pa")
                    for t in range(TPB):
                        nc.tensor.matmul(psum_blk[:], oh[:, t, :], msg[:, t, :],
                                         start=(t == 0), stop=(t == TPB - 1),
                                         skip_group_check=True)
                    absb = st.tile([128, F160], bf16, name="absb", tag="absb")
                    nc.scalar.copy(absb[:], psum_blk[:])
                    bcols = slice(b * 128, (b + 1) * 128)
                    pt1 = psT.tile([128, 128], bf16, name="pt1", tag="pt")
                    nc.tensor.transpose(pt1[:], absb[:, 0:128], identb[:])
                    nc.scalar.copy(AT1[:, bcols], pt1[:])
                    pt2 = psT.tile([32, 128], bf16, name="pt2", tag="pt")
                    nc.tensor.transpose(pt2[:], absb[:, 128:160], identb[:])
                    nc.scalar.copy(AT2[:, bcols], pt2[:])

                NCHK = (W + 511) // 512
                for cc in range(NCHK):
                    cs = slice(cc * 512, min((cc + 1) * 512, W))
                    ncols = cs.stop - cs.start
                    pw = psW.tile([128, 512], f32, name="pw", tag="pw", bufs=2)
                    pwb = psW.tile([32, 512], f32, name="pwb", tag="pwb", bufs=1)
                    prep_ = psW.tile([128, 512], f32, name="prep_", tag="prep_",
                                     bufs=1)
                    nc.tensor.matmul(pw[:, :ncols],
                                     abwD_t[:, l * 128:(l + 1) * 128],
                                     AT1[:, cs])
                    nc.tensor.matmul(pwb[:, :ncols],
                                     abwB_t[:, l * 32:(l + 1) * 32],
                                     AT2[:, cs])
                    a0sb = st.tile([16, 512], bf16, name="a0sb", tag="a0sb")
                    nc.scalar.copy(a0sb[:, :ncols], pw[0:16, :ncols])
                    nc.tensor.matmul(prep_[:, :ncols], rep_t[:], a0sb[:, :ncols])
                    nc.vector.tensor_scalar(Fa[:, cs], prep_[:, :ncols],
                                            wsva[:, l * 5 + 1:l * 5 + 2],
                                            wsva[:, l * 5 + 0:l * 5 + 1],
                                            op0=ALU.mult, op1=ALU.add)
                    nc.vector.tensor_scalar(Fb[:, cs], prep_[0:32, :ncols],
                                            wsvb[:, l * 5 + 1:l * 5 + 2],
                                            wsvb[:, l * 5 + 0:l * 5 + 1],
                                            op0=ALU.mult, op1=ALU.add)
                    nc.vector.tensor_tensor(Fa[:, cs], Fa[:, cs], pw[:, :ncols],
                                            ALU.mult)
                    nc.vector.tensor_tensor(Fb[:, cs], Fb[:, cs], pwb[:, :ncols],
                                            ALU.mult)
                if l == 0:
                    nc.vector.tensor_scalar(hTa[:], Fa[:], wsva[:, 3:4],
                                            wsva[:, 4:5], op0=ALU.mult, op1=ALU.add)
                    nc.vector.tensor_scalar(hTb[:], Fb[:], wsvb[:, 3:4],
                                            wsvb[:, 4:5], op0=ALU.mult, op1=ALU.add)
                else:
                    nc.vector.tensor_scalar(hTa[:], hTa[:],
                                            wsva[:, l * 5 + 2:l * 5 + 3],
                                            None, op0=ALU.mult)
                    nc.vector.scalar_tensor_tensor(hTa[:], Fa[:],
                                                   wsva[:, l * 5 + 3:l * 5 + 4],
                                                   hTa[:], ALU.mult, ALU.add)
                    nc.vector.tensor_scalar(hTb[:], hTb[:],
                                            wsvb[:, l * 5 + 2:l * 5 + 3],
                                            None, op0=ALU.mult)
                    nc.vector.scalar_tensor_tensor(hTb[:], Fb[:],
                                                   wsvb[:, l * 5 + 3:l * 5 + 4],
                                                   hTb[:], ALU.mult, ALU.add)

                if l == 0:
                    for b in range(B):
                        bcols = slice(b * 128, (b + 1) * 128)
                        pta = psT.tile([128, 128], bf16, name="pta", tag="pt")
                        nc.tensor.transpose(pta[:], hTa[:, bcols], identb[:])
                        ptb2 = psT.tile([128, 32], bf16, name="ptb2", tag="pt")
                        nc.tensor.transpose(ptb2[:], hTb[:, bcols],
                                            identb[0:32, 0:32])
                        hsb = st.tile([128, FPAD], bf16, name="hsb", tag="hsb")
                        nc.scalar.copy(hsb[:, 0:128], pta[:])
                        nc.scalar.copy(hsb[:, 128:160], ptb2[:])
                        nc.gpsimd.memset(hsb[:, 160:FPAD], 0.0)
                        nc.sync.dma_start(h_local[b * 128:(b + 1) * 128, :], hsb[:])
                    if sim_mode:
                        nc.sync.dma_start(h_full[0:B * 128, :], h_local[:])
                    else:
                        nc.gpsimd.collective_compute(
                            "AllGather", ALU.bypass,
                            replica_groups=[list(range(n_cores))],
                            ins=[h_local.opt()], outs=[h_full.opt()])

            # ---- pooling + head ----
            psum_pool = psA.tile([64, F160], f32, name="psum_pool", tag="pa")
            for b in range(B):
                bcols = slice(b * 128, (b + 1) * 128)
                pta = psT.tile([128, 128], bf16, name="pta", tag="pt")
                nc.tensor.transpose(pta[:], hTa[:, bcols], identb[:])
                ptb2 = psT.tile([128, 32], bf16, name="ptb2", tag="pt")
                nc.tensor.transpose(ptb2[:], hTb[:, bcols], identb[0:32, 0:32])
                flat = st.tile([128, F160], bf16, name="flat", tag="flat")
                nc.scalar.copy(flat[:, 0:128], pta[:])
                nc.scalar.copy(flat[:, 128:160], ptb2[:])
                goh = st.tile([128, 64], bf16, name="goh", tag="goh")
                nc.vector.tensor_scalar(goh[:], iota_bf[:, 0:64],
                                        batch_t[:, b:b + 1], None, op0=ALU.is_equal)
                nc.tensor.matmul(psum_pool[:], goh[:], flat[:],
                                 start=(b == 0), stop=(b == B - 1),
                                 skip_group_check=True)
            pooled = st.tile([64, F160], f32, name="pooled", tag="pooled")
            nc.scalar.copy(pooled[:], psum_pool[:])
            nc.sync.dma_start(ar_in[:], pooled[:])
            if sim_mode:
                nc.sync.dma_start(ar_out[:], ar_in[:])
            else:
                nc.gpsimd.collective_compute(
                    "AllReduce", ALU.add,
                    replica_groups=[list(range(n_cores))],
                    ins=[ar_in.opt()], outs=[ar_out.opt()])
            pooled_ar = st.tile([64, F160], f32, name="pooled_ar", tag="pooled_ar")
            nc.sync.dma_start(pooled_ar[:], ar_out[:])
            pt1f = psT.tile([128, 128], f32, name="pt1f", tag="pt")
            nc.tensor.transpose(pt1f[:, 0:64], pooled_ar[:, 0:128], identf[:])
            pt2f = psT.tile([32, 64], f32, name="pt2f", tag="pt")
            nc.tensor.transpose(pt2f[:], pooled_ar[:, 128:160], identf[:])
            pt1s = st.tile([128, 64], f32, name="pt1s", tag="pt1s")
            nc.scalar.copy(pt1s[:], pt1f[:, 0:64])
            pt2s = st.tile([32, 64], f32, name="pt2s", tag="pt2s")
            nc.scalar.copy(pt2s[:], pt2f[:])
            pso = psA.tile([64, 2], f32, name="pso", tag="pa")
            nc.tensor.matmul(pso[:], pt1s[:], pw1[:], start=True, stop=False)
            nc.tensor.matmul(pso[:], pt2s[:], pw2[:], start=False, stop=True)
            outsb = st.tile([64, 2], f32, name="outsb", tag="outsb")
            nc.vector.tensor_tensor(outsb[:], pso[:], pb[:], ALU.add)
            nc.sync.dma_start(out_t[:], outsb[:])

    nc.compile()
    return nc


def _run_spmd(nc, in_maps, n_cores):
    import jax
    from jax.sharding import Mesh, PartitionSpec
    from jax.experimental.shard_map import shard_map
    from concourse.bass2jax import (_bass_exec_p, install_neuronx_cc_hook,
                                    partition_id_tensor)
    install_neuronx_cc_hook()
    partition_name = nc.partition_id_tensor.name if nc.partition_id_tensor else None
    in_names, out_names, out_avals, zero_outs = [], [], [], []
    for alloc in nc.m.functions[0].allocations:
        if not isinstance(alloc, mybir.MemoryLocationSet):
            continue
        name = alloc.memorylocations[0].name
        if alloc.kind == "ExternalInput":
            if name != partition_name:
                in_names.append(name)
        elif alloc.kind == "ExternalOutput":
            shape, dt = alloc.tensor_shape, mybir.dt.np(alloc.dtype)
            out_names.append(name)
            out_avals.append(jax.core.ShapedArray(shape, dt))
            zero_outs.append(np.zeros(shape, dt))
    all_in_names = in_names + out_names + ([partition_name] if partition_name else [])

    def _body(*args):
        operands = list(args)
        if partition_name is not None:
            operands.append(partition_id_tensor())
        outs = _bass_exec_p.bind(
            *operands, out_avals=tuple(out_avals), in_names=tuple(all_in_names),
            out_names=tuple(out_names), lowering_input_output_aliases=(),
            sim_require_finite=False, sim_require_nnan=False, nc=nc)
        return tuple(outs)

    devices = jax.devices()[:n_cores]
    mesh = Mesh(np.asarray(devices), ("core",))
    n_params, n_outs = len(in_names), len(out_names)
    fn = jax.jit(
        shard_map(_body, mesh=mesh,
                  in_specs=(PartitionSpec("core"),) * (n_params + n_outs),
                  out_specs=(PartitionSpec("core"),) * n_outs, check_rep=False),
        keep_unused=True)
    concat_in = [
        np.concatenate([np.asarray(in_maps[c][nm]) for c in range(n_cores)], axis=0)
        for nm in in_names]
    concat_zero = [np.zeros((n_cores * z.shape[0], *z.shape[1:]), z.dtype)
                   for z in zero_outs]
    outs = fn(*concat_in, *concat_zero)
    res = {}
    for i, nm in enumerate(out_names):
        res[nm] = np.asarray(outs[i]).reshape(n_cores, *out_avals[i].shape)[0]
    return res


def kernel(pos, emb_weight, ab_w, ws_w, channel_w, message_w, pred_W, pred_b,
           atoms, edge_index, batch):
    pos = np.asarray(pos, dtype=np.float32)
    emb_weight = np.asarray(emb_weight, dtype=np.float32)
    ab_w = np.asarray(ab_w, dtype=np.float32)
    ws_w = np.asarray(ws_w, dtype=np.float32)
    channel_w = np.asarray(channel_w, dtype=np.float32)
    message_w = np.asarray(message_w, dtype=np.float32)
    pred_W = np.asarray(pred_W, dtype=np.float32)
    pred_b = np.asarray(pred_b, dtype=np.float32)
    edge_index = np.asarray(edge_index)
    batch = np.asarray(batch)

    prep = _host_prep(pos, emb_weight, ab_w, ws_w, channel_w, message_w,
                      pred_W, pred_b, atoms, edge_index, batch)
    nc = _build(prep)
    B = prep["B"]
    in_maps = []
    for k in range(NCORES):
        in_maps.append({
            "pos_pad": prep["pos_pad"],
            "npi": prep["npi"],
            "pos_idx": prep["pos_idx"][k],
            "slot_idxb": prep["slot_idxb"][k],
            "rv_dev": np.ascontiguousarray(prep["rv_dev"][k].reshape(128, B * TPB)),
            "batch_slot": prep["batch_slot"][k],
            "abwD": prep["abwD"],
            "abwB": prep["abwB"],
            "rep16": prep["rep16"],
            "wsv_a": prep["wsv_a"],
            "wsv_b": prep["wsv_b"],
            "predW1": prep["predW1"],
            "predW2": prep["predW2"],
            "predb": prep["predb_rep"],
        })
    res = _run_spmd(nc, in_maps, NCORES)
    return res["out"].astype(np.float32)


# revision 13
# speedup vs baseline: 2.0211x; 1.0073x over previous
"""CartesianMACE message-passing kernel for 8 Trainium2 NeuronCores.

Self-contained: kernel(**inputs) takes the FULL inputs and returns the FULL
[G, OUT] output. Edges are sharded by receiver across 8 cores
(receiver-contiguous blocks of <=128 nodes / <=2048 edges); a Bass/Tile SPMD
kernel does segment-sum via one-hot matmuls into PSUM, node-level channel
mixing on the PE, an AllGather of node features between the two layers, and
per-graph pooling + AllReduce + prediction head on-device.

The compute core runs in bf16 (messages, one-hots, matmuls, channel mixing)
with fp32 geometry (distances, radial embedding) and fp32 accumulation in
PSUM; host-side work is index/layout preparation only.
"""
import sys
sys.path.insert(0, "/opt/trn_rl_repo")
import numpy as np
import concourse.bass as bass
import concourse.bacc as bacc
import concourse.mybir as mybir
from concourse import tile, masks

f32 = mybir.dt.float32
bf16 = mybir.dt.bfloat16
i16 = mybir.dt.int16
i32 = mybir.dt.int32
AF = mybir.ActivationFunctionType
ALU = mybir.AluOpType

PI = float(np.pi)
TWO_PI = float(2 * np.pi)
MAGIC = 12582912.0  # 1.5*2^23: add/sub rounds fp32 to nearest int
CW1 = float(np.float32(6.28125))
CW2 = float(np.float32(TWO_PI - CW1))
CW3 = float(TWO_PI - CW1 - float(np.float32(TWO_PI - CW1)))
INV2PI = float(1.0 / TWO_PI)

NCORES = 8
C = 16
L = 2
RANKS = 3
EPB = 2048
TPB = 16
BLKN = 128
F160 = 160
FPAD = 256  # bf16 h row: 512B, gather-friendly
POSPAD = 64
KMAP = [0, 1, 1, 1, 2, 2, 2, 2, 2, 2]
SYM_A = np.array([0, 0, 0, 1, 1, 2])
SYM_B = np.array([0, 1, 2, 1, 2, 2])


def _wrap_idx(idx, cols):
    n = idx.shape[0]
    w = np.zeros((16, cols), dtype=np.int16)
    w[np.arange(n) % 16, np.arange(n) // 16] = idx.astype(np.int16)
    return np.tile(w, (8, 1))


def _host_prep(pos, emb_weight, ab_w, ws_w, channel_w, message_w, pred_W, pred_b,
               atoms, edge_index, batch):
    N = pos.shape[0]
    OUT = pred_W.shape[1]
    src = np.asarray(edge_index[0], dtype=np.int64)
    rcv = np.asarray(edge_index[1], dtype=np.int64)

    deg = np.bincount(rcv, minlength=N)
    assert deg.max() <= EPB

    blocks = []
    n0, cur = 0, 0
    for n in range(N):
        d = deg[n]
        if n > n0 and (n - n0 >= BLKN or cur + d > EPB):
            blocks.append((n0, n))
            n0, cur = n, 0
        cur += d
    blocks.append((n0, N))
    B = (len(blocks) + NCORES - 1) // NCORES
    while len(blocks) < NCORES * B:
        blocks.append((N, N))
    NSLOT = NCORES * B * BLKN
    assert NSLOT < 32768  # int16 gather indices

    node_slot = np.full(N, -1, dtype=np.int64)
    for bi, (a, b) in enumerate(blocks):
        node_slot[a:b] = bi * BLKN + np.arange(b - a)

    order = np.argsort(rcv, kind="stable")
    rs, ss = rcv[order], src[order]
    starts = np.searchsorted(rs, [a for a, _ in blocks])
    ends = np.searchsorted(rs, [b for _, b in blocks])

    esnd = np.zeros((NCORES, B, EPB), dtype=np.int64)
    ercv = np.ones((NCORES, B, EPB), dtype=np.int64)
    eslot = np.zeros((NCORES, B, EPB), dtype=np.int64)
    rv_adj = np.full((NCORES, B, EPB), -1.0, dtype=np.float32)
    for bi, (a, b) in enumerate(blocks):
        k, bb = divmod(bi, B)
        e0, e1 = starts[bi], ends[bi]
        ne = e1 - e0
        esnd[k, bb, :ne] = ss[e0:e1]
        ercv[k, bb, :ne] = rs[e0:e1]
        eslot[k, bb, :ne] = node_slot[ss[e0:e1]]
        rv_adj[k, bb, :ne] = (rs[e0:e1] - a).astype(np.float32)

    rv_dev = np.transpose(rv_adj.reshape(NCORES, B, TPB, 128), (0, 3, 1, 2)).copy()

    pos_idx = np.zeros((NCORES, B * 128, 256), dtype=np.int16)
    slot_idxb = np.zeros((NCORES, 128, B * 128), dtype=np.int16)
    for k in range(NCORES):
        for bb in range(B):
            tok = np.concatenate([esnd[k, bb], ercv[k, bb]])
            pos_idx[k, bb * 128:(bb + 1) * 128, :] = _wrap_idx(tok, 256)
            slot_idxb[k, :, bb * 128:(bb + 1) * 128] = _wrap_idx(eslot[k, bb], 128)

    batch_slot = np.full((NCORES, 128, B), -1.0, dtype=np.float32)
    for bi, (a, b) in enumerate(blocks):
        k, bb = divmod(bi, B)
        batch_slot[k, :b - a, bb] = batch[a:b].astype(np.float32)

    pos_pad = np.zeros((N, POSPAD), dtype=np.float32)
    pos_pad[:, :3] = pos
    npi = np.tile((np.arange(1, C + 1) * PI).astype(np.float32)[None, :], (128, 1))

    emb0 = np.asarray(emb_weight[0], dtype=np.float32)
    abwT = np.zeros((L, RANKS, C, C), dtype=np.float32)
    for c in range(RANKS):
        abwT[0, c] = (ab_w[0, c] * emb0[None, :]).T
        abwT[1, c] = (ab_w[1, c] * (2.0 if c == 0 else 1.0)).T
    # block-diagonal packing: feats 0:128 = groups 0..7, feats 128:160 = 8,9
    abwD = np.zeros((L, 128, 128), dtype=np.float32)
    abwB = np.zeros((L, 32, 32), dtype=np.float32)
    for l in range(L):
        for g in range(8):
            abwD[l, g * 16:(g + 1) * 16, g * 16:(g + 1) * 16] = abwT[l, KMAP[g]]
        for g in range(2):
            abwB[l, g * 16:(g + 1) * 16, g * 16:(g + 1) * 16] = abwT[l, KMAP[8 + g]]
    abwD_dev = np.transpose(abwD, (1, 0, 2)).reshape(128, L * 128)
    abwB_dev = np.transpose(abwB, (1, 0, 2)).reshape(32, L * 32)
    rep16 = np.zeros((16, 128), dtype=np.float32)
    rep16[np.arange(128) % 16, np.arange(128)] = 1.0

    def vec_for(l, fn):
        v = np.zeros(160, dtype=np.float32)
        for k in range(10):
            v[k * 16:(k + 1) * 16] = fn(l, KMAP[k])
        return v
    wsv = np.zeros((L, 160, 5), dtype=np.float32)
    for l in range(L):
        wsv[l, :, 0] = vec_for(l, lambda l_, c: ws_w[l_, 0, c])
        wsv[l, :, 1] = vec_for(l, lambda l_, c: ws_w[l_, 1, c])
        wsv[l, :, 2] = vec_for(l, lambda l_, c: channel_w[l_, c].sum(0))
        wsv[l, :, 3] = vec_for(l, lambda l_, c: message_w[l_, c].sum(0))
    wsv[0, 0:16, 4] = emb0 * channel_w[0, 0].sum(0)
    wsv_a = np.transpose(wsv[:, 0:128, :], (1, 0, 2)).reshape(128, L * 5).copy()
    wsv_b = np.transpose(wsv[:, 128:160, :], (1, 0, 2)).reshape(32, L * 5).copy()

    predW_eff = np.zeros((160, OUT), dtype=np.float32)
    predW_eff[0:16] = pred_W[0:16]
    for x in range(3):
        for ch in range(C):
            predW_eff[16 + x * 16 + ch] = pred_W[16 + ch * 3 + x]
    for s in range(6):
        a_, b_ = SYM_A[s], SYM_B[s]
        for ch in range(C):
            w = pred_W[64 + ch * 9 + a_ * 3 + b_].copy()
            if a_ != b_:
                w = w + pred_W[64 + ch * 9 + b_ * 3 + a_]
            predW_eff[64 + s * 16 + ch] = w
    predb_rep = np.tile(np.asarray(pred_b, dtype=np.float32)[None, :], (64, 1))

    def as_bf(x):
        import ml_dtypes
        return np.asarray(x, dtype=ml_dtypes.bfloat16)

    return dict(N=N, B=B, NSLOT=NSLOT, pos_idx=pos_idx, slot_idxb=slot_idxb,
                rv_dev=rv_dev, batch_slot=batch_slot, pos_pad=pos_pad, npi=npi,
                abwD=as_bf(abwD_dev), abwB=as_bf(abwB_dev), rep16=as_bf(rep16),
                wsv_a=wsv_a, wsv_b=wsv_b,
                predW1=predW_eff[0:128].copy(), predW2=predW_eff[128:160].copy(),
                predb_rep=predb_rep)


def _build(prep, n_cores=NCORES, sim_mode=False):
    B = prep["B"]
    NSLOT = prep["NSLOT"]
    W = B * 128

    nc = bacc.Bacc("TRN2", target_bir_lowering=False, debug=False,
                   num_devices=1 if sim_mode else n_cores)

    pos_pad = nc.dram_tensor("pos_pad", [prep["N"], POSPAD], f32, kind="ExternalInput")
    npi_in = nc.dram_tensor("npi", [128, C], f32, kind="ExternalInput")
    pidx_in = nc.dram_tensor("pos_idx", [B * 128, 256], i16, kind="ExternalInput")
    slot_in = nc.dram_tensor("slot_idxb", [128, B * 128], i16, kind="ExternalInput")
    rv_in = nc.dram_tensor("rv_dev", [128, B * TPB], f32, kind="ExternalInput")
    batch_in = nc.dram_tensor("batch_slot", [128, B], f32, kind="ExternalInput")
    abwD_in = nc.dram_tensor("abwD", [128, L * 128], bf16, kind="ExternalInput")
    abwB_in = nc.dram_tensor("abwB", [32, L * 32], bf16, kind="ExternalInput")
    rep_in = nc.dram_tensor("rep16", [16, 128], bf16, kind="ExternalInput")
    wsva_in = nc.dram_tensor("wsv_a", [128, L * 5], f32, kind="ExternalInput")
    wsvb_in = nc.dram_tensor("wsv_b", [32, L * 5], f32, kind="ExternalInput")
    pw1_in = nc.dram_tensor("predW1", [128, 2], f32, kind="ExternalInput")
    pw2_in = nc.dram_tensor("predW2", [32, 2], f32, kind="ExternalInput")
    pb_in = nc.dram_tensor("predb", [64, 2], f32, kind="ExternalInput")
    out_t = nc.dram_tensor("out", [64, 2], f32, kind="ExternalOutput")

    with tile.TileContext(nc) as tc:
        with (
            tc.tile_pool(name="const", bufs=1) as cpool,
            tc.tile_pool(name="geo", bufs=1) as geo,
            tc.tile_pool(name="st", bufs=2) as st,
            tc.tile_pool(name="wide", bufs=1) as wd,
            tc.tile_pool(name="psA", bufs=2, space="PSUM") as psA,
            tc.tile_pool(name="psT", bufs=1, space="PSUM") as psT,
            tc.tile_pool(name="psW", bufs=1, space="PSUM") as psW,
            tc.tile_pool(name="dram", bufs=1, space="DRAM") as dram,
        ):
            def ct(shape, name, dt=f32):
                return cpool.tile(shape, dt, name=name, tag=name)

            npi = ct([128, C], "npi_t")
            nc.sync.dma_start(npi[:], npi_in[:])
            rv_all = ct([128, B * TPB], "rv_all")
            nc.sync.dma_start(rv_all[:], rv_in[:])
            batch_t = ct([128, B], "batch_t")
            nc.sync.dma_start(batch_t[:], batch_in[:])
            abwD_t = ct([128, L * 128], "abwD_t", bf16)
            nc.sync.dma_start(abwD_t[:], abwD_in[:])
            abwB_t = ct([32, L * 32], "abwB_t", bf16)
            nc.sync.dma_start(abwB_t[:], abwB_in[:])
            rep_t = ct([16, 128], "rep_t", bf16)
            nc.sync.dma_start(rep_t[:], rep_in[:])
            wsva = ct([128, L * 5], "wsva_t")
            nc.sync.dma_start(wsva[:], wsva_in[:])
            wsvb = ct([32, L * 5], "wsvb_t")
            nc.sync.dma_start(wsvb[:], wsvb_in[:])
            pw1 = ct([128, 2], "pw1_t")
            nc.sync.dma_start(pw1[:], pw1_in[:])
            pw2 = ct([32, 2], "pw2_t")
            nc.sync.dma_start(pw2[:], pw2_in[:])
            pb = ct([64, 2], "pb_t")
            nc.sync.dma_start(pb[:], pb_in[:])
            iota_i = ct([128, 128], "iota_i", i32)
            nc.gpsimd.iota(iota_i[:], pattern=[[1, 128]], base=0, channel_multiplier=0)
            iota_bf = ct([128, 128], "iota_bf", bf16)
            nc.vector.tensor_copy(iota_bf[:], iota_i[:])
            identb = ct([128, 128], "identb", bf16)
            masks.make_identity(nc, identb[:])
            identf = ct([64, 64], "identf")
            masks.make_identity(nc, identf[:])

            # per-block persistent tiles
            mblk = [geo.tile([128, TPB, F160], bf16, name=f"mblk{b}", tag=f"mblk{b}")
                    for b in range(B)]
            relb = [geo.tile([128, TPB, 3], f32, name=f"relb{b}", tag=f"relb{b}")
                    for b in range(B)]
            d2_all = geo.tile([128, B * TPB], f32, name="d2_all", tag="d2_all")
            d_all = geo.tile([128, B * TPB], f32, name="d_all", tag="d_all")
            invd_all = geo.tile([128, B * TPB], f32, name="invd_all", tag="invd_all")

            def wt(shape, name, dt=bf16):
                return wd.tile(shape, dt, name=name, tag=name)
            AT1 = wt([128, W], "AT1")
            AT2 = wt([32, W], "AT2")
            Fa = AT1   # dead after the pw matmul of each chunk; reuse in place
            Fb = AT2
            hTa = wt([128, W], "hTa")
            hTb = wt([32, W], "hTb")

            h_local = dram.tile([B * 128, FPAD], bf16, name="h_local", tag="h_local")
            if sim_mode:
                h_full = dram.tile([NSLOT, FPAD], bf16, name="h_full", tag="h_full")
            else:
                h_full = dram.tile([NSLOT, FPAD], bf16, name="h_full", tag="h_full",
                                   addr_space="Shared")
            ar_in = dram.tile([64, F160], f32, name="ar_in", tag="ar_in")
            ar_out = dram.tile([64, F160], f32, name="ar_out", tag="ar_out")

            def seg_block(l, b, msg):
                oh = st.tile([128, TPB, 128], bf16, name="oh", tag="oh", bufs=2)
                for t in range(TPB):
                    nc.vector.tensor_scalar(
                        oh[:, t, :], iota_bf[:],
                        rv_all[:, b * TPB + t:b * TPB + t + 1], None,
                        op0=ALU.is_equal)
                psum_blk = psA.tile([128, F160], f32, name="psum_blk", tag="pa")
                for t in range(TPB):
                    nc.tensor.matmul(psum_blk[:], oh[:, t, :], msg[:, t, :],
                                     start=(t == 0), stop=(t == TPB - 1),
                                     skip_group_check=True)
                absb = st.tile([128, F160], bf16, name="absb", tag="absb")
                nc.scalar.copy(absb[:], psum_blk[:])
                bcols = slice(b * 128, (b + 1) * 128)
                pt1 = psT.tile([128, 128], bf16, name="pt1", tag="pt")
                nc.tensor.transpose(pt1[:], absb[:, 0:128], identb[:])
                nc.scalar.copy(AT1[:, bcols], pt1[:])
                pt2 = psT.tile([32, 128], bf16, name="pt2", tag="pt")
                nc.tensor.transpose(pt2[:], absb[:, 128:160], identb[:])
                nc.scalar.copy(AT2[:, bcols], pt2[:])

            # ---- geometry (grouped sqrt) fused with layer-0 blocks ----
            GRP = 5
            for g0 in range(0, B, GRP):
                gblocks = range(g0, min(g0 + GRP, B))
                for b in gblocks:
                    pidx = st.tile([128, 256], i16, name="pidx", tag="pidx")
                    nc.sync.dma_start(pidx[:], pidx_in[b * 128:(b + 1) * 128, :])
                    pp = st.tile([128, 32, POSPAD], f32, name="pp", tag="pp", bufs=2, )
                    for qk in range(4):
                        nc.gpsimd.dma_gather(pp[:, 8 * qk:8 * (qk + 1), :],
                                             pos_pad[:, :],
                                             pidx[:, 64 * qk:64 * (qk + 1)],
                                             num_idxs=1024, num_idxs_reg=1024,
                                             elem_size=POSPAD)
                    bsl = slice(b * TPB, (b + 1) * TPB)
                    nc.vector.tensor_tensor(relb[b][:], pp[:, 0:TPB, 0:3],
                                            pp[:, TPB:32, 0:3], ALU.subtract)
                    sq = st.tile([128, TPB, 3], f32, name="sq", tag="sq")
                    nc.vector.tensor_tensor(sq[:], relb[b][:], relb[b][:], ALU.mult)
                    nc.vector.reduce_sum(d2_all[:, bsl], sq[:],
                                         axis=mybir.AxisListType.X)
                gsl = slice(g0 * TPB, min(g0 + GRP, B) * TPB)
                nc.scalar.sqrt(d_all[:, gsl], d2_all[:, gsl])
                nc.vector.reciprocal(invd_all[:, gsl], d_all[:, gsl])
                for b in gblocks:
                    bsl = slice(b * TPB, (b + 1) * TPB)
                    invb3 = invd_all[:, bsl].unsqueeze(2).broadcast_to([128, TPB, 3])
                    nc.vector.tensor_tensor(relb[b][:], relb[b][:], invb3, ALU.mult)
                    ruu9 = st.tile([128, TPB, 9], f32, name="ruu9", tag="ruu9",
                                   bufs=2)
                    nc.vector.tensor_copy(ruu9[:, :, 0:3], relb[b][:])
                    o = 3
                    for a_ in range(3):
                        n_ = 3 - a_
                        nc.vector.tensor_tensor(
                            ruu9[:, :, o:o + n_],
                            relb[b][:, :, a_:a_ + 1].broadcast_to([128, TPB, n_]),
                            relb[b][:, :, a_:3], ALU.mult)
                        o += n_
                    args = st.tile([128, TPB, C], f32, name="args", tag="args",
                                   bufs=1)
                    nc.vector.tensor_tensor(
                        args[:],
                        npi[:].unsqueeze(1).broadcast_to([128, TPB, C]),
                        d_all[:, bsl].unsqueeze(2).broadcast_to([128, TPB, C]),
                        ALU.mult)
                    tk = st.tile([128, TPB, C], f32, name="tk", tag="tk", bufs=1)
                    nc.vector.tensor_scalar(tk[:], args[:], INV2PI, MAGIC,
                                            op0=ALU.mult, op1=ALU.add)
                    nc.vector.tensor_scalar(tk[:], tk[:], MAGIC, None,
                                            op0=ALU.subtract)
                    red = st.tile([128, TPB, C], f32, name="red", tag="red", bufs=1)
                    nc.vector.cody_waite_cascade(
                        red[:].rearrange("p t c -> p (t c)"),
                        args[:].rearrange("p t c -> p (t c)"),
                        tk[:].rearrange("p t c -> p (t c)"),
                        CW1, CW2, CW3)
                    sn = st.tile([128, TPB, C], f32, name="sn", tag="sn", bufs=2)
                    nc.scalar.activation(sn[:], red[:], AF.Sin)
                    nc.vector.tensor_tensor(
                        mblk[b][:, :, 0:16], sn[:],
                        invd_all[:, bsl].unsqueeze(2).broadcast_to([128, TPB, C]),
                        ALU.mult)
                    nc.vector.tensor_tensor(
                        mblk[b][:, :, 16:160].rearrange("p t (s c) -> p t s c", c=C),
                        ruu9[:].unsqueeze(3).broadcast_to([128, TPB, 9, C]),
                        mblk[b][:, :, 0:16].unsqueeze(2).broadcast_to(
                            [128, TPB, 9, C]),
                        ALU.mult)

            psum_pool = psA.tile([64, F160], f32, name="psum_pool", tag="pool")
            # ---- layers ----
            for l in range(L):
                if l == 0:
                    for b in range(B):
                        seg_block(0, b, mblk[b])
                else:
                    for b in range(B):
                        slot_pb = st.tile([128, 128], i16, name="slot_pb",
                                          tag="slot_pb", bufs=3)
                        nc.sync.dma_start(slot_pb[:],
                                          slot_in[:, b * 128:(b + 1) * 128])
                        hgb = st.tile([128, TPB, FPAD], bf16, name="hgb", tag="hgb",
                                      bufs=2)
                        for qk in range(2):
                            nc.gpsimd.dma_gather(
                                hgb[:, 8 * qk:8 * (qk + 1), :], h_full[:, :],
                                slot_pb[:, 64 * qk:64 * (qk + 1)],
                                num_idxs=1024, num_idxs_reg=1024,
                                elem_size=FPAD)
                        msg = st.tile([128, TPB, F160], bf16, name="msg",
                                      tag="msg", bufs=2)
                        nc.vector.tensor_tensor(
                            msg[:].rearrange("p t (s c) -> p t s c", c=C),
                            mblk[b][:].rearrange("p t (s c) -> p t s c", c=C),
                            hgb[:, :, 0:16].unsqueeze(2).broadcast_to(
                                [128, TPB, 10, C]),
                            ALU.mult)
                        tmp = st.tile([128, TPB, 144], bf16, name="tmp",
                                      tag="tmp", bufs=2)
                        nc.vector.tensor_tensor(
                            tmp[:].rearrange("p t (s c) -> p t s c", c=C),
                            hgb[:, :, 16:160].rearrange("p t (s c) -> p t s c", c=C),
                            mblk[b][:, :, 0:16].unsqueeze(2).broadcast_to(
                                [128, TPB, 9, C]),
                            ALU.mult)
                        nc.vector.tensor_tensor(msg[:, :, 16:160], msg[:, :, 16:160],
                                                tmp[:], ALU.add)
                        seg_block(l, b, msg)
                NCHK = (W + 511) // 512
                for cc in range(NCHK):
                    cs = slice(cc * 512, min((cc + 1) * 512, W))
                    ncols = cs.stop - cs.start
                    pw = psW.tile([128, 512], f32, name="pw", tag="pw", bufs=1)
                    pwb = psW.tile([32, 512], f32, name="pwb", tag="pwb", bufs=1)
                    prep_ = psW.tile([128, 512], f32, name="prep_", tag="prep_",
                                     bufs=1)
                    nc.tensor.matmul(pw[:, :ncols],
                                     abwD_t[:, l * 128:(l + 1) * 128],
                                     AT1[:, cs])
                    nc.tensor.matmul(pwb[:, :ncols],
                                     abwB_t[:, l * 32:(l + 1) * 32],
                                     AT2[:, cs])
                    a0sb = st.tile([16, 512], bf16, name="a0sb", tag="a0sb")
                    nc.scalar.copy(a0sb[:, :ncols], pw[0:16, :ncols])
                    nc.tensor.matmul(prep_[:, :ncols], rep_t[:], a0sb[:, :ncols])
                    nc.vector.tensor_scalar(Fa[:, cs], prep_[:, :ncols],
                                            wsva[:, l * 5 + 1:l * 5 + 2],
                                            wsva[:, l * 5 + 0:l * 5 + 1],
                                            op0=ALU.mult, op1=ALU.add)
                    nc.vector.tensor_scalar(Fb[:, cs], prep_[0:32, :ncols],
                                            wsvb[:, l * 5 + 1:l * 5 + 2],
                                            wsvb[:, l * 5 + 0:l * 5 + 1],
                                            op0=ALU.mult, op1=ALU.add)
                    nc.vector.tensor_tensor(Fa[:, cs], Fa[:, cs], pw[:, :ncols],
                                            ALU.mult)
                    nc.vector.tensor_tensor(Fb[:, cs], Fb[:, cs], pwb[:, :ncols],
                                            ALU.mult)
                    bpc = 512 // 128
                    blks = range(cc * bpc, min((cc + 1) * bpc, B))
                    if l == 0:
                        nc.vector.tensor_scalar(hTa[:, cs], Fa[:, cs], wsva[:, 3:4],
                                                wsva[:, 4:5], op0=ALU.mult,
                                                op1=ALU.add)
                        nc.vector.tensor_scalar(hTb[:, cs], Fb[:, cs], wsvb[:, 3:4],
                                                wsvb[:, 4:5], op0=ALU.mult,
                                                op1=ALU.add)
                        for b in blks:
                            bcols = slice(b * 128, (b + 1) * 128)
                            pta = psT.tile([128, 128], bf16, name="pta", tag="pt")
                            nc.tensor.transpose(pta[:], hTa[:, bcols], identb[:])
                            ptb2 = psT.tile([128, 32], bf16, name="ptb2", tag="pt")
                            nc.tensor.transpose(ptb2[:], hTb[:, bcols],
                                                identb[0:32, 0:32])
                            hsb = st.tile([128, FPAD], bf16, name="hsb", tag="hsb")
                            nc.scalar.copy(hsb[:, 0:128], pta[:])
                            nc.scalar.copy(hsb[:, 128:160], ptb2[:])
                            nc.gpsimd.memset(hsb[:, 160:FPAD], 0.0)
                            nc.sync.dma_start(h_local[b * 128:(b + 1) * 128, :],
                                              hsb[:])
                    else:
                        nc.vector.tensor_scalar(hTa[:, cs], hTa[:, cs],
                                                wsva[:, l * 5 + 2:l * 5 + 3],
                                                None, op0=ALU.mult)
                        nc.vector.scalar_tensor_tensor(hTa[:, cs], Fa[:, cs],
                                                       wsva[:, l * 5 + 3:l * 5 + 4],
                                                       hTa[:, cs], ALU.mult, ALU.add)
                        nc.vector.tensor_scalar(hTb[:, cs], hTb[:, cs],
                                                wsvb[:, l * 5 + 2:l * 5 + 3],
                                                None, op0=ALU.mult)
                        nc.vector.scalar_tensor_tensor(hTb[:, cs], Fb[:, cs],
                                                       wsvb[:, l * 5 + 3:l * 5 + 4],
                                                       hTb[:, cs], ALU.mult, ALU.add)
                        for b in blks:
                            bcols = slice(b * 128, (b + 1) * 128)
                            pta = psT.tile([128, 128], bf16, name="pta", tag="pt")
                            nc.tensor.transpose(pta[:], hTa[:, bcols], identb[:])
                            ptb2 = psT.tile([128, 32], bf16, name="ptb2", tag="pt")
                            nc.tensor.transpose(ptb2[:], hTb[:, bcols],
                                                identb[0:32, 0:32])
                            flat = st.tile([128, F160], bf16, name="flat",
                                           tag="flat")
                            nc.scalar.copy(flat[:, 0:128], pta[:])
                            nc.scalar.copy(flat[:, 128:160], ptb2[:])
                            goh = st.tile([128, 64], bf16, name="goh", tag="goh")
                            nc.vector.tensor_scalar(goh[:], iota_bf[:, 0:64],
                                                    batch_t[:, b:b + 1], None,
                                                    op0=ALU.is_equal)
                            nc.tensor.matmul(psum_pool[:], goh[:], flat[:],
                                             start=(b == 0), stop=(b == B - 1),
                                             skip_group_check=True)
                if l == 0:
                    if sim_mode:
                        nc.sync.dma_start(h_full[0:B * 128, :], h_local[:])
                    else:
                        nc.gpsimd.collective_compute(
                            "AllGather", ALU.bypass,
                            replica_groups=[list(range(n_cores))],
                            ins=[h_local.opt()], outs=[h_full.opt()])

            # ---- head ----
            pooled = st.tile([64, F160], f32, name="pooled", tag="pooled")
            nc.scalar.copy(pooled[:], psum_pool[:])
            nc.sync.dma_start(ar_in[:], pooled[:])
            if sim_mode:
                nc.sync.dma_start(ar_out[:], ar_in[:])
            else:
                nc.gpsimd.collective_compute(
                    "AllReduce", ALU.add,
                    replica_groups=[list(range(n_cores))],
                    ins=[ar_in.opt()], outs=[ar_out.opt()])
            pooled_ar = st.tile([64, F160], f32, name="pooled_ar", tag="pooled_ar")
            nc.sync.dma_start(pooled_ar[:], ar_out[:])
            pt1f = psT.tile([128, 128], f32, name="pt1f", tag="pt")
            nc.tensor.transpose(pt1f[:, 0:64], pooled_ar[:, 0:128], identf[:])
            pt2f = psT.tile([32, 64], f32, name="pt2f", tag="pt")
            nc.tensor.transpose(pt2f[:], pooled_ar[:, 128:160], identf[:])
            pt1s = st.tile([128, 64], f32, name="pt1s", tag="pt1s")
            nc.scalar.copy(pt1s[:], pt1f[:, 0:64])
            pt2s = st.tile([32, 64], f32, name="pt2s", tag="pt2s")
            nc.scalar.copy(pt2s[:], pt2f[:])
            pso = psA.tile([64, 2], f32, name="pso", tag="pa")
            nc.tensor.matmul(pso[:], pt1s[:], pw1[:], start=True, stop=False)
            nc.tensor.matmul(pso[:], pt2s[:], pw2[:], start=False, stop=True)
            outsb = st.tile([64, 2], f32, name="outsb", tag="outsb")
            nc.vector.tensor_tensor(outsb[:], pso[:], pb[:], ALU.add)
            nc.sync.dma_start(out_t[:], outsb[:])

    nc.compile()
    return nc


def _run_spmd(nc, in_maps, n_cores):
    import jax
    from jax.sharding import Mesh, PartitionSpec
    from jax.experimental.shard_map import shard_map
    from concourse.bass2jax import (_bass_exec_p, install_neuronx_cc_hook,
                                    partition_id_tensor)
    install_neuronx_cc_hook()
    partition_name = nc.partition_id_tensor.name if nc.partition_id_tensor else None
    in_names, out_names, out_avals, zero_outs = [], [], [], []
    for alloc in nc.m.functions[0].allocations:
        if not isinstance(alloc, mybir.MemoryLocationSet):
            continue
        name = alloc.memorylocations[0].name
        if alloc.kind == "ExternalInput":
            if name != partition_name:
                in_names.append(name)
        elif alloc.kind == "ExternalOutput":
            shape, dt = alloc.tensor_shape, mybir.dt.np(alloc.dtype)
            out_names.append(name)
            out_avals.append(jax.core.ShapedArray(shape, dt))
            zero_outs.append(np.zeros(shape, dt))
    all_in_names = in_names + out_names + ([partition_name] if partition_name else [])

    def _body(*args):
        operands = list(args)
        if partition_name is not None:
            operands.append(partition_id_tensor())
        outs = _bass_exec_p.bind(
            *operands, out_avals=tuple(out_avals), in_names=tuple(all_in_names),
            out_names=tuple(out_names), lowering_input_output_aliases=(),
            sim_require_finite=False, sim_require_nnan=False, nc=nc)
        return tuple(outs)

    devices = jax.devices()[:n_cores]
    mesh = Mesh(np.asarray(devices), ("core",))
    n_params, n_outs = len(in_names), len(out_names)
    fn = jax.jit(
        shard_map(_body, mesh=mesh,
                  in_specs=(PartitionSpec("core"),) * (n_params + n_outs),
                  out_specs=(PartitionSpec("core"),) * n_outs, check_rep=False),
        keep_unused=True)
    concat_in = [
        np.concatenate([np.asarray(in_maps[c][nm]) for c in range(n_cores)], axis=0)
        for nm in in_names]
    concat_zero = [np.zeros((n_cores * z.shape[0], *z.shape[1:]), z.dtype)
                   for z in zero_outs]
    outs = fn(*concat_in, *concat_zero)
    res = {}
    for i, nm in enumerate(out_names):
        res[nm] = np.asarray(outs[i]).reshape(n_cores, *out_avals[i].shape)[0]
    return res


def kernel(pos, emb_weight, ab_w, ws_w, channel_w, message_w, pred_W, pred_b,
           atoms, edge_index, batch):
    pos = np.asarray(pos, dtype=np.float32)
    emb_weight = np.asarray(emb_weight, dtype=np.float32)
    ab_w = np.asarray(ab_w, dtype=np.float32)
    ws_w = np.asarray(ws_w, dtype=np.float32)
    channel_w = np.asarray(channel_w, dtype=np.float32)
    message_w = np.asarray(message_w, dtype=np.float32)
    pred_W = np.asarray(pred_W, dtype=np.float32)
    pred_b = np.asarray(pred_b, dtype=np.float32)
    edge_index = np.asarray(edge_index)
    batch = np.asarray(batch)

    prep = _host_prep(pos, emb_weight, ab_w, ws_w, channel_w, message_w,
                      pred_W, pred_b, atoms, edge_index, batch)
    nc = _build(prep)
    B = prep["B"]
    in_maps = []
    for k in range(NCORES):
        in_maps.append({
            "pos_pad": prep["pos_pad"],
            "npi": prep["npi"],
            "pos_idx": prep["pos_idx"][k],
            "slot_idxb": prep["slot_idxb"][k],
            "rv_dev": np.ascontiguousarray(prep["rv_dev"][k].reshape(128, B * TPB)),
            "batch_slot": prep["batch_slot"][k],
            "abwD": prep["abwD"],
            "abwB": prep["abwB"],
            "rep16": prep["rep16"],
            "wsv_a": prep["wsv_a"],
            "wsv_b": prep["wsv_b"],
            "predW1": prep["predW1"],
            "predW2": prep["predW2"],
            "predb": prep["predb_rep"],
        })
    res = _run_spmd(nc, in_maps, NCORES)
    return res["out"].astype(np.float32)
